# revision 1
# baseline (speedup 1.0000x reference)
"""Edge-parallel GNN message passing on 8 Trainium2 NeuronCores.

Strategy (host-permuted, fully core-independent):
  * Sort edges by destination node. Pack whole destination segments into
    128-edge tiles (padding so no segment spans a tile). Each tile owns a
    disjoint set of destination nodes; tiles are dealt contiguously to the
    8 cores -> no collective needed.
  * Per 128-edge tile, on device:
      stage 1: 32 fp32 matmuls, each computing 4 edges' (x_src @ A_e) via a
               block-diagonal x operand (K=128 = 4 edges x 32 dims):
               msgT[32f, 4e] = A_block[128,32].T-contract x_block[128,4].
      transpose msgT [32,128] -> msg [128,32] on the PE (identity matmul).
      stage 2: segment-sum via one-hot selector matmul S.T @ msg, where
               S[e, m] = (rank[e] == m) is built on-device (DVE is_equal
               against an iota tile). Slot ranks and 1/count come from host.
      epilogue: mean = sum * recip (ACT per-partition scale), + bias
               (GPSIMD), relu (ACT).
  * DMAs are batched over super-tiles of SB edge-tiles (HWDGE descriptor
    generation costs ~625ns per dma_start, so instruction count matters).
  * Host scatters the per-(tile,slot) rows to node ids; isolated nodes get
    relu(bias).

The 2 GB a_in stream dominates: ~256 MB/core fp32, fully sequential.
"""

import math
import os
from contextlib import ExitStack

import numpy as np

import concourse.bass as bass  # noqa: F401
import concourse.tile as tile
from concourse import bacc, mybir
from concourse.bass_utils import run_bass_kernel_spmd

F32 = mybir.dt.float32
NCORES = 8
D = 32
EPT = 128          # edges per tile
GPT = EPT // 4     # stage-1 matmul groups per tile
SB = 4             # edge-tiles per super-tile (DMA batch)
OG = 2             # super-tiles per output DMA


def _pack_segments(counts):
    """Greedy-pack whole segments (each <= EPT) into EPT-slot tiles."""
    n = len(counts)
    tile_id = np.empty(n, np.int64)
    slot = np.empty(n, np.int64)
    t = 0
    used = 0
    nseg = 0
    for i in range(n):
        c = counts[i]
        if used + c > EPT:
            t += 1
            used = 0
            nseg = 0
        tile_id[i] = t
        slot[i] = nseg
        used += c
        nseg += 1
    return tile_id, slot, (t + 1 if n else 0)


def _prep(node_states, edge_index, a_in, bias):
    ns = np.asarray(node_states, dtype=np.float32)
    ei = np.asarray(edge_index)
    a = np.asarray(a_in, dtype=np.float32)
    b = np.asarray(bias, dtype=np.float32)
    n_nodes, d = ns.shape
    assert d == D
    src = np.ascontiguousarray(ei[:, 0]).astype(np.int64)
    dst = np.ascontiguousarray(ei[:, 1]).astype(np.int64)

    perm = np.argsort(dst, kind="stable")
    dsts = dst[perm]
    nodes_u, counts = np.unique(dsts, return_counts=True)

    # Oversize segments (in-degree > EPT) fall back to host compute.
    big = counts > EPT
    host_nodes = nodes_u[big]
    edge_big = np.repeat(big, counts)
    perm_k = perm[~edge_big]
    nodes_k = nodes_u[~big]
    counts_k = counts[~big]

    tile_id, slot, n_tiles = _pack_segments(counts_k)
    n_tiles = max(n_tiles, 1)
    TS = int(math.ceil(n_tiles / (NCORES * SB)))   # super-tiles per core
    TS = int(math.ceil(TS / OG)) * OG              # whole output groups
    T = TS * SB                                    # edge-tiles per core
    Ttot = T * NCORES

    ek = len(perm_k)
    if ek:
        e_tile = np.repeat(tile_id, counts_k)
        cum_excl = np.concatenate(([0], np.cumsum(counts_k)))[:-1]
        tile_first_seg = np.searchsorted(tile_id, np.arange(n_tiles))
        tile_edge_start = cum_excl[tile_first_seg]
        e_pos = np.arange(ek) - tile_edge_start[e_tile]
        flat = e_tile * EPT + e_pos
    else:
        flat = np.zeros(0, np.int64)

    ei_flat = np.zeros(Ttot * EPT, np.int64)
    if ek:
        ei_flat[flat] = perm_k
    rank_flat = np.full(Ttot * EPT, -1e9, np.float32)
    recip_flat = np.ones(Ttot * EPT, np.float32)
    flatslot = tile_id * EPT + slot
    if ek:
        rank_flat[flat] = np.repeat(slot, counts_k).astype(np.float32)
        recip_flat[flatslot] = (1.0 / counts_k).astype(np.float32)

    # One fused device stream per super-tile (single DMA): per partition row
    # p = 32j+d the columns are
    #   [0            , SB*1024)  A2[t',p, 1024s+32g+f] = a[e(t,s,g,j),d,f]
    #   [SB*1024      , +SB*32 )  Xc[t',p, 32s+g]       = x_src[e(..)][d]
    #   [SB*1024+SB*32, +2*SB  )  rr (rank, recip) per tile s at 2s, 2s+1
    #                             (only meaningful on partitions = edge slot)
    AW = SB * GPT * D
    XW = SB * GPT
    AXRW = AW + XW + 2 * SB
    AXR_host = np.empty((NCORES, TS, 128, AXRW), np.float32)
    ei_r = ei_flat.reshape(NCORES, T * EPT)
    xsrc = src[ei_flat].reshape(NCORES, T * EPT)
    rank_r = rank_flat.reshape(NCORES, TS, SB, EPT)
    recip_r = recip_flat.reshape(NCORES, TS, SB, EPT)
    for c in range(NCORES):
        ae = a[ei_r[c]]                                   # [T*EPT, D, D]
        AXR_host[c, :, :, :AW] = (
            ae.reshape(TS, SB, GPT, 4, D, D)
            .transpose(0, 3, 4, 1, 2, 5)                  # [t', j, d, s, g, f]
            .reshape(TS, 128, AW)
        )
        del ae
        xg = ns[xsrc[c]]                                  # [T*EPT, D]
        AXR_host[c, :, :, AW:AW + XW] = (
            xg.reshape(TS, SB, GPT, 4, D)
            .transpose(0, 3, 4, 1, 2)                     # [t', j, d, s, g]
            .reshape(TS, 128, XW)
        )
        del xg
        rr = np.stack([rank_r[c], recip_r[c]], axis=-1)   # [t', s, p, 2]
        AXR_host[c, :, :, AW + XW:] = rr.transpose(0, 2, 1, 3).reshape(
            TS, EPT, 2 * SB
        )

    iota_host = np.tile(np.arange(128, dtype=np.float32), (128, 1))
    ident_host = np.eye(32, dtype=np.float32)
    biasbc_host = np.tile(b, (128, 1)).astype(np.float32)

    in_maps = [
        {
            "axr": AXR_host[c],
            "iota": iota_host,
            "ident": ident_host,
            "biasbc": biasbc_host,
        }
        for c in range(NCORES)
    ]

    host_rows = None
    if len(host_nodes):
        eb = perm[edge_big]
        msg = np.einsum("ed,edf->ef", ns[src[eb]], a[eb])
        summed = np.zeros((len(host_nodes), D), np.float32)
        hn_index = {n: i for i, n in enumerate(host_nodes)}
        idx = np.fromiter((hn_index[n] for n in dst[eb]), np.int64, len(eb))
        np.add.at(summed, idx, msg)
        cnt = counts[big].astype(np.float32)[:, None]
        host_rows = np.maximum(summed / cnt + b[None, :], 0.0).astype(np.float32)

    meta = dict(
        n_nodes=n_nodes,
        TS=TS,
        nodes_k=nodes_k,
        flatslot=flatslot,
        host_nodes=host_nodes,
        host_rows=host_rows,
        bias=b,
    )
    return in_maps, meta


def _build(TS, enable_asserts=False, repeat=1):
    nc = bacc.Bacc(
        "TRN2",
        target_bir_lowering=False,
        debug=False,
        enable_asserts=enable_asserts,
        num_devices=NCORES,
    )
    AW = SB * GPT * D
    XW = SB * GPT
    AXRW = AW + XW + 2 * SB
    axr_d = nc.dram_tensor("axr", [TS, 128, AXRW], F32, kind="ExternalInput")
    iota_d = nc.dram_tensor("iota", [128, 128], F32, kind="ExternalInput")
    id_d = nc.dram_tensor("ident", [32, 32], F32, kind="ExternalInput")
    bb_d = nc.dram_tensor("biasbc", [128, 32], F32, kind="ExternalInput")
    TSo = (TS + OG - 1) // OG
    out_d = nc.dram_tensor("out", [TSo, EPT, OG * SB * D], F32, kind="ExternalOutput")

    with tile.TileContext(nc) as tc, ExitStack() as ctx:
        cpool = ctx.enter_context(tc.tile_pool(name="const", bufs=1))
        apool = ctx.enter_context(tc.tile_pool(name="apool", bufs=3))
        spool = ctx.enter_context(tc.tile_pool(name="spool", bufs=3))
        wpool = ctx.enter_context(tc.tile_pool(name="wpool", bufs=4))
        opool = ctx.enter_context(tc.tile_pool(name="opool", bufs=3))
        ps_a = ctx.enter_context(tc.tile_pool(name="ps_a", bufs=2, space="PSUM"))
        ps_b = ctx.enter_context(tc.tile_pool(name="ps_b", bufs=2, space="PSUM"))
        ps_c = ctx.enter_context(tc.tile_pool(name="ps_c", bufs=2, space="PSUM"))

        iota_t = cpool.tile([128, 128], F32, tag="iota")
        nc.sync.dma_start(iota_t[:], iota_d[:])
        id_t = cpool.tile([32, 32], F32, tag="ident")
        nc.sync.dma_start(id_t[:], id_d[:])
        bb_t = cpool.tile([128, 32], F32, tag="biasbc")
        nc.sync.dma_start(bb_t[:], bb_d[:])

        # Two persistent block-diagonal x operands (one per parity); the
        # off-diagonal cells are zeroed once and never rewritten (DMAs only
        # touch the diagonal 32x32 blocks), so reuse keeps them zero.
        xm = []
        for i in range(2):
            t_ = cpool.tile([128, 128 * SB], F32, tag=f"xmega{i}")
            nc.vector.memset(t_[:], 0.0)
            xm.append(t_)

        for tp in [tt for _ in range(repeat) for tt in range(TS)]:
            at = apool.tile([128, AXRW], F32, tag="a")
            nc.sync.dma_start(at[:], axr_d[tp])

            # Spread the compact x columns into the block-diagonal operand:
            # same partitions, column-only moves (DVE-legal). Off-diagonal
            # blocks of x_mega stay zero from the one-time memset.
            x_mega = xm[tp % 2]
            xv4 = x_mega.rearrange("p (s j g) -> p s j g", s=SB, j=4)
            xc = at[:, AW : AW + XW].rearrange("p (s g) -> p s g", s=SB)
            for j in range(4):
                nc.vector.tensor_copy(
                    xv4[32 * j : 32 * j + 32, :, j, :],
                    xc[32 * j : 32 * j + 32],
                )
            rrt = at[:, AW + XW :]

            if tp % OG == 0:
                osup = opool.tile([128, OG * SB * D], F32, tag="o")
                if tp + OG > TS:
                    # final partial group: zero the never-written columns
                    nc.vector.memset(osup[:], 0.0)
            oc = (tp % OG) * SB * D

            for s in range(SB):
                msgT_ps = ps_a.tile([32, 128], F32, tag="msgT")
                for g in range(GPT):
                    nc.tensor.matmul(
                        msgT_ps[:, 4 * g : 4 * g + 4],
                        at[:, 1024 * s + 32 * g : 1024 * s + 32 * g + 32],
                        xv4[:, s, :, g],
                        start=True,
                        stop=True,
                    )
                msgT_sb = wpool.tile([32, 128], F32, tag="msgTsb")
                nc.scalar.copy(msgT_sb[:], msgT_ps[:])

                msg_ps = ps_b.tile([128, 32], F32, tag="msg")
                nc.tensor.transpose(msg_ps[:], msgT_sb[:], id_t[:])
                msg_sb = wpool.tile([128, 32], F32, tag="msgsb")
                nc.vector.tensor_copy(msg_sb[:], msg_ps[:])

                s_t = spool.tile([128, 128], F32, tag="S")
                nc.vector.tensor_scalar(
                    s_t[:],
                    iota_t[:],
                    rrt[:, 2 * s : 2 * s + 1],
                    None,
                    mybir.AluOpType.is_equal,
                )

                sum_ps = ps_c.tile([128, 32], F32, tag="sum")
                nc.tensor.matmul(sum_ps[:], s_t[:], msg_sb[:], start=True, stop=True)

                mean_sb = wpool.tile([128, 32], F32, tag="mean")
                nc.scalar.activation(
                    mean_sb[:],
                    sum_ps[:],
                    mybir.ActivationFunctionType.Copy,
                    bias=0.0,
                    scale=rrt[:, 2 * s + 1 : 2 * s + 2],
                )
                pb_sb = wpool.tile([128, 32], F32, tag="pb")
                nc.gpsimd.tensor_add(pb_sb[:], mean_sb[:], bb_t[:])
                nc.scalar.activation(
                    osup[:, oc + D * s : oc + D * s + D],
                    pb_sb[:],
                    mybir.ActivationFunctionType.Relu,
                )

            if tp % OG == OG - 1 or tp == TS - 1:
                nc.sync.dma_start(out_d[tp // OG], osup[:])

    nc.compile()
    return nc


_BUILD_CACHE = {}


def _built(TS):
    nc = _BUILD_CACHE.get(TS)
    if nc is None:
        nc = _build(TS)
        _BUILD_CACHE[TS] = nc
    return nc


def _finalize(results, meta):
    sup = np.concatenate([r["out"] for r in results], axis=0)  # [NC*TSo,EPT,OG*SB*D]
    ncts = sup.shape[0]
    rows = (
        sup.reshape(ncts, EPT, -1, D)
        .transpose(0, 2, 1, 3)                                 # [t'', og*s, p, f]
        .reshape(-1, D)
    )
    b = meta["bias"]
    out = np.empty((meta["n_nodes"], D), np.float32)
    out[:] = np.maximum(b, 0.0)[None, :]
    out[meta["nodes_k"]] = rows[meta["flatslot"]]
    if meta["host_rows"] is not None:
        out[meta["host_nodes"]] = meta["host_rows"]
    return out


def kernel(node_states, edge_index, a_in, bias):
    in_maps, meta = _prep(node_states, edge_index, a_in, bias)
    nc = _built(meta["TS"])
    res = run_bass_kernel_spmd(nc, in_maps, list(range(NCORES)))
    return _finalize(res.results, meta)


if __name__ == "__main__":
    np.random.seed(0)
    n_nodes, n_edges = 700, 3000
    ns = np.random.randn(n_nodes, D).astype(np.float32)
    ei = np.random.randint(0, n_nodes, (n_edges, 2)).astype(np.int64)
    a = (np.random.randn(n_edges, D, D) / np.sqrt(D)).astype(np.float32)
    b = np.random.uniform(-0.2, 0.2, D).astype(np.float32)

    x_i = ns[ei[:, 0]]
    msg = np.einsum("ed,edf->ef", x_i, a)
    summed = np.zeros((n_nodes, D), np.float32)
    np.add.at(summed, ei[:, 1], msg)
    cnt = np.bincount(ei[:, 1], minlength=n_nodes).astype(np.float32)
    expected = np.maximum(summed / np.maximum(cnt, 1.0)[:, None] + b[None, :], 0.0)

    if os.environ.get("RUN_HW"):
        actual = kernel(ns, ei, a, b)
    else:
        from concourse.bass_interp import CoreSim

        in_maps, meta = _prep(ns, ei, a, b)
        nc = _build(meta["TS"], enable_asserts=True)
        outs = []
        for c in range(NCORES):
            sim = CoreSim(nc, trace=False)
            for k, v in in_maps[c].items():
                sim.tensor(k)[:] = v
            sim.simulate()
            outs.append({"out": np.array(sim.tensor("out"))})
        actual = _finalize(outs, meta)

    err = np.abs(actual - expected)
    denom = np.abs(expected).max()
    print("max abs err:", err.max(), "rel to scale:", err.max() / denom)
    rel = np.linalg.norm(actual - expected) / np.linalg.norm(expected)
    print("l2 rel:", rel)
    assert err.max() / denom < 1e-4, "FAIL"
    print("PASS")



# revision 12
# speedup vs baseline: 27.3530x; 27.3530x over previous
"""Edge-parallel GNN message passing on 8 Trainium2 NeuronCores.

Strategy (host-permuted, fully core-independent, bf16 stream):
  * Sort edges by destination node. Pack whole destination segments into
    128-edge tiles (max 32 segments per tile, so segment sums fit the 32
    output partitions of one PSUM tile). Tiles are dealt contiguously to
    the 8 cores -> no collective needed.
  * Per 128-edge tile, on device:
      stage 1: 32 bf16 matmuls, each computing 4 edges' (x_src @ A_e) via
               a block-diagonal x operand (K=128 = 4 edges x 32 dims):
               msgT[32f, 4e] = A_block[128,32].T-contract x_block[128,4].
      DVE stream-transpose flips each 32x32 block of msgT [32,128] so
               chunk b holds msg rows for edges 32b..32b+32 on
               partitions 0-31.
      stage 2: 4 accumulating K=32 matmuls with a recip-weighted one-hot
               selector S'[e',m] = (slot(e')==m)/count built on-device
               (DVE tensor_scalar is_equal*mult against an iota tile),
               yielding the segment MEAN directly; plus one K=1 rank-1
               matmul ones[1,32] x bias[1,32] that adds bias to every
               slot row. Epilogue is a single ACT Relu.
  * A/x/metadata are streamed as ONE fused bf16 DMA per 8-tile
    super-tile (the 2 GB a_in stream dominates; bf16 halves it).
  * Host scatters the per-(tile,slot) rows to node ids; isolated nodes
    get relu(bias); in-degree > 128 nodes fall back to host compute.
"""

import math
import os
from contextlib import ExitStack

import numpy as np
import ml_dtypes

import concourse.bass as bass  # noqa: F401
import concourse.tile as tile
from concourse import bacc, mybir
from concourse.bass_utils import run_bass_kernel_spmd

F32 = mybir.dt.float32
BF16 = mybir.dt.bfloat16
NPBF16 = ml_dtypes.bfloat16
NCORES = 8
D = 32
EPT = 128          # edges per tile
GPT = EPT // 4     # stage-1 matmul groups per tile
NSLOT = 32         # max destination segments per tile
SB = 8             # edge-tiles per super-tile (one fused DMA each)
ACOLS = GPT * D    # 1024
XCOLS = GPT        # 32
TCOLS = ACOLS + XCOLS  # 1056 bf16 columns per tile
MCOLS = 8          # rank4 | recip4 (separate f32 stream)
PAD_RANK = -1.0e9


def _pack_segments(counts):
    """Greedy-pack whole segments (each <= EPT edges) into tiles holding
    at most EPT edges and NSLOT segments."""
    n = len(counts)
    tile_id = np.empty(n, np.int64)
    slot = np.empty(n, np.int64)
    t = 0
    used = 0
    nseg = 0
    for i in range(n):
        c = counts[i]
        if used + c > EPT or nseg == NSLOT:
            t += 1
            used = 0
            nseg = 0
        tile_id[i] = t
        slot[i] = nseg
        used += c
        nseg += 1
    return tile_id, slot, (t + 1 if n else 0)


def _prep(node_states, edge_index, a_in, bias):
    ns = np.asarray(node_states, dtype=np.float32)
    ei = np.asarray(edge_index)
    a = np.asarray(a_in, dtype=np.float32)
    b = np.asarray(bias, dtype=np.float32)
    n_nodes, d = ns.shape
    assert d == D
    src = np.ascontiguousarray(ei[:, 0]).astype(np.int64)
    dst = np.ascontiguousarray(ei[:, 1]).astype(np.int64)

    perm = np.argsort(dst, kind="stable")
    dsts = dst[perm]
    nodes_u, counts = np.unique(dsts, return_counts=True)

    # Oversize segments (in-degree > EPT) fall back to host compute.
    big = counts > EPT
    host_nodes = nodes_u[big]
    edge_big = np.repeat(big, counts)
    perm_k = perm[~edge_big]
    nodes_k = nodes_u[~big]
    counts_k = counts[~big]

    tile_id, slot, n_tiles = _pack_segments(counts_k)
    n_tiles = max(n_tiles, 1)
    TS = int(math.ceil(n_tiles / (NCORES * SB)))   # super-tiles per core
    T = TS * SB                                    # edge-tiles per core
    Ttot = T * NCORES

    ek = len(perm_k)
    if ek:
        e_tile = np.repeat(tile_id, counts_k)
        cum_excl = np.concatenate(([0], np.cumsum(counts_k)))[:-1]
        tile_first_seg = np.searchsorted(tile_id, np.arange(n_tiles))
        tile_edge_start = cum_excl[tile_first_seg]
        e_pos = np.arange(ek) - tile_edge_start[e_tile]
        flat = e_tile * EPT + e_pos
    else:
        flat = np.zeros(0, np.int64)

    ei_flat = np.zeros(Ttot * EPT, np.int64)
    rank_flat = np.full(Ttot * EPT, PAD_RANK, np.float32)
    recip_flat = np.zeros(Ttot * EPT, np.float32)
    if ek:
        ei_flat[flat] = perm_k
        rank_flat[flat] = np.repeat(slot, counts_k).astype(np.float32)
        recip_flat[flat] = np.repeat(
            (1.0 / counts_k).astype(np.float32), counts_k
        )
    flatslot = tile_id * NSLOT + slot

    # One fused bf16 device stream per super-tile: [128, SB, TCOLS] where
    # per tile s the columns are
    #   [0,    1024)  A:  [p=32j+d, 32g+f] = a[e(4g+j), d, f]
    #   [1024, 1056)  x:  [p=32j+d, g]     = ns[src(e(4g+j)), d]
    # plus a small f32 metadata stream [32, SB, 8] per super-tile:
    #   [., s, b]     rank4  [e'<32]       = slot(e=32b+e') or -1e9 pad
    #   [., s, 4+b]   recip4 [e'<32]       = 1/count(segment of e) or 0
    AXR_host = np.zeros((NCORES, TS, 128, SB, TCOLS), NPBF16)
    META_host = np.zeros((NCORES, TS, 32, SB, MCOLS), np.float32)
    ei_r = ei_flat.reshape(NCORES, T * EPT)
    xsrc = src[ei_flat].reshape(NCORES, T * EPT)
    rank_r = rank_flat.reshape(NCORES, TS, SB, 4, 32)
    recip_r = recip_flat.reshape(NCORES, TS, SB, 4, 32)
    for c in range(NCORES):
        ae = a[ei_r[c]]                                   # [T*EPT, D, D]
        AXR_host[c, :, :, :, :ACOLS] = (
            ae.reshape(TS, SB, GPT, 4, D, D)
            .transpose(0, 3, 4, 1, 2, 5)                  # [t', j, d, s, g, f]
            .reshape(TS, 128, SB, ACOLS)
        )
        del ae
        xg = ns[xsrc[c]]                                  # [T*EPT, D]
        AXR_host[c, :, :, :, ACOLS:ACOLS + XCOLS] = (
            xg.reshape(TS, SB, GPT, 4, D)
            .transpose(0, 3, 4, 1, 2)                     # [t', j, d, s, g]
            .reshape(TS, 128, SB, XCOLS)
        )
        del xg
        META_host[c, :, :, :, :4] = rank_r[c].transpose(0, 3, 1, 2)
        META_host[c, :, :, :, 4:] = recip_r[c].transpose(0, 3, 1, 2)

    iota_host = np.tile(np.arange(NSLOT, dtype=np.float32), (32, 1))
    ob_host = np.zeros((1, 64), NPBF16)
    ob_host[0, :32] = 1.0
    ob_host[0, 32:] = b

    in_maps = [
        {
            "axr": AXR_host[c].reshape(TS, 128, SB * TCOLS),
            "meta": META_host[c].reshape(TS, 32, SB * MCOLS),
            "iota": iota_host,
            "onesbias": ob_host,
        }
        for c in range(NCORES)
    ]

    host_rows = None
    if len(host_nodes):
        eb = perm[edge_big]
        msg = np.einsum("ed,edf->ef", ns[src[eb]], a[eb])
        summed = np.zeros((len(host_nodes), D), np.float32)
        hn_index = {n: i for i, n in enumerate(host_nodes)}
        idx = np.fromiter((hn_index[n] for n in dst[eb]), np.int64, len(eb))
        np.add.at(summed, idx, msg)
        cnt = counts[big].astype(np.float32)[:, None]
        host_rows = np.maximum(summed / cnt + b[None, :], 0.0).astype(np.float32)

    meta = dict(
        n_nodes=n_nodes,
        TS=TS,
        nodes_k=nodes_k,
        flatslot=flatslot,
        host_nodes=host_nodes,
        host_rows=host_rows,
        bias=b,
    )
    return in_maps, meta


def _build(TS, enable_asserts=False):
    nc = bacc.Bacc(
        "TRN2",
        target_bir_lowering=False,
        debug=False,
        enable_asserts=enable_asserts,
        num_devices=NCORES,
    )
    axr_d = nc.dram_tensor("axr", [TS, 128, SB * TCOLS], BF16, kind="ExternalInput")
    meta_d = nc.dram_tensor("meta", [TS, 32, SB * MCOLS], F32, kind="ExternalInput")
    iota_d = nc.dram_tensor("iota", [32, NSLOT], F32, kind="ExternalInput")
    ob_d = nc.dram_tensor("onesbias", [1, 64], BF16, kind="ExternalInput")
    out_d = nc.dram_tensor("out", [TS, NSLOT, SB * D], F32, kind="ExternalOutput")

    with tile.TileContext(nc) as tc, ExitStack() as ctx:
        cpool = ctx.enter_context(tc.tile_pool(name="const", bufs=1))
        apool = ctx.enter_context(tc.tile_pool(name="apool", bufs=3))
        spool = ctx.enter_context(tc.tile_pool(name="spool", bufs=3))
        wpool = ctx.enter_context(tc.tile_pool(name="wpool", bufs=4))
        opool = ctx.enter_context(tc.tile_pool(name="opool", bufs=2))
        ps_m = ctx.enter_context(tc.tile_pool(name="ps_m", bufs=2, space="PSUM"))
        ps_s = ctx.enter_context(tc.tile_pool(name="ps_s", bufs=2, space="PSUM"))

        iota_t = cpool.tile([32, NSLOT], F32, tag="iota")
        nc.sync.dma_start(iota_t[:], iota_d[:])
        ob_t = cpool.tile([1, 64], BF16, tag="ob")
        nc.sync.dma_start(ob_t[:], ob_d[:])

        # Two persistent block-diagonal x operands (one per super parity);
        # off-diagonal cells zeroed once (spread copies only touch the
        # diagonal 32-row blocks, so reuse keeps them zero).
        xm = []
        for i in range(2):
            t_ = cpool.tile([128, SB, 4, GPT], BF16, tag=f"xmega{i}")
            nc.vector.memset(t_[:], 0.0)
            xm.append(t_)

        def dma_in(q):
            t_ = apool.tile([128, SB * TCOLS], BF16, tag="axr")
            nc.sync.dma_start(t_[:], axr_d[q])
            m_ = spool.tile([32, SB * MCOLS], F32, tag="meta")
            nc.sync.dma_start(m_[:], meta_d[q])
            return t_, m_

        at_buf = {0: dma_in(0)}
        if TS > 1:
            at_buf[1] = dma_in(1)

        pend = None  # (s_t, msgTT, osup, s) of the previous tile

        def finish(pend_v):
            s_t, msgTT, osup_, s_ = pend_v
            sum_ps = ps_s.tile([NSLOT, D], F32, tag="sum")
            for bk in range(4):
                nc.tensor.matmul(
                    sum_ps[:],
                    s_t[:, bk, :],
                    msgTT[:, 32 * bk : 32 * bk + 32],
                    start=(bk == 0),
                    stop=False,
                )
            nc.tensor.matmul(
                sum_ps[:], ob_t[:, 0:32], ob_t[:, 32:64], start=False, stop=True
            )
            nc.scalar.activation(
                osup_[:, D * s_ : D * s_ + D],
                sum_ps[:],
                mybir.ActivationFunctionType.Relu,
            )

        osup = None
        for q in range(TS):
            at, mt = at_buf.pop(q)
            if q + 2 < TS:
                at_buf[q + 2] = dma_in(q + 2)
            atv = at.rearrange("p (s c) -> p s c", s=SB)
            xv = xm[q % 2]

            # Spread compact x columns into the block-diagonal operand
            # (column-only moves within each 32-partition slab; split
            # across DVE and ACT to balance engine load).
            for j in range(4):
                dst_ = xv[32 * j : 32 * j + 32, :, j, :]
                src_ = atv[32 * j : 32 * j + 32, :, ACOLS : ACOLS + XCOLS]
                if j < 2:
                    nc.vector.tensor_copy(dst_, src_)
                else:
                    nc.scalar.copy(dst_, src_)

            osup_prev = osup
            osup = opool.tile([NSLOT, SB * D], F32, tag="osup")

            for s in range(SB):
                # stage 1: msgT[32f, 128e] in PSUM
                msgT_ps = ps_m.tile([32, EPT], F32, tag="msgT")
                for g in range(GPT):
                    nc.tensor.matmul(
                        msgT_ps[:, 4 * g : 4 * g + 4],
                        atv[:, s, 32 * g : 32 * g + 32],
                        xv[:, s, :, g],
                        start=True,
                        stop=True,
                    )
                # selector S'[e', b, m] = (slot==m) * recip  (bf16)
                s_t = spool.tile([32, 4, NSLOT], BF16, tag="S")
                for bk in range(4):
                    nc.vector.tensor_scalar(
                        s_t[:, bk, :],
                        iota_t[:],
                        mt[:, MCOLS * s + bk : MCOLS * s + bk + 1],
                        mt[:, MCOLS * s + 4 + bk : MCOLS * s + 5 + bk],
                        mybir.AluOpType.is_equal,
                        mybir.AluOpType.mult,
                    )
                # PSUM -> SBUF (bf16) then 32x32 block transpose on DVE
                msgT_sb = wpool.tile([32, EPT], BF16, tag="msgTsb")
                nc.scalar.copy(msgT_sb[:], msgT_ps[:])
                msgTT = wpool.tile([32, EPT], BF16, tag="msgTT")
                nc.vector.transpose(msgTT[:], msgT_sb[:])

                if pend is not None:
                    finish(pend)
                    if s == 0 and q > 0:
                        nc.sync.dma_start(out_d[q - 1], osup_prev[:])
                pend = (s_t, msgTT, osup, s)

        finish(pend)
        nc.sync.dma_start(out_d[TS - 1], osup[:])

    nc.compile()
    return nc


_BUILD_CACHE = {}


def _built(TS):
    nc = _BUILD_CACHE.get(TS)
    if nc is None:
        nc = _build(TS)
        _BUILD_CACHE[TS] = nc
    return nc


def _finalize(results, meta):
    sup = np.concatenate([r["out"] for r in results], axis=0)  # [NC*TS,32,SB*D]
    ncts = sup.shape[0]
    rows = (
        sup.reshape(ncts, NSLOT, SB, D)
        .transpose(0, 2, 1, 3)                                 # [t', s, m, f]
        .reshape(-1, D)
    )
    b = meta["bias"]
    out = np.empty((meta["n_nodes"], D), np.float32)
    out[:] = np.maximum(b, 0.0)[None, :]
    out[meta["nodes_k"]] = rows[meta["flatslot"]]
    if meta["host_rows"] is not None:
        out[meta["host_nodes"]] = meta["host_rows"]
    return out


def kernel(node_states, edge_index, a_in, bias):
    in_maps, meta = _prep(node_states, edge_index, a_in, bias)
    nc = _built(meta["TS"])
    res = run_bass_kernel_spmd(nc, in_maps, list(range(NCORES)))
    return _finalize(res.results, meta)


if __name__ == "__main__":
    np.random.seed(0)
    n_nodes, n_edges = 700, 3000
    ns = np.random.randn(n_nodes, D).astype(np.float32)
    ei = np.random.randint(0, n_nodes, (n_edges, 2)).astype(np.int64)
    a = (np.random.randn(n_edges, D, D) / np.sqrt(D)).astype(np.float32)
    b = np.random.uniform(-0.2, 0.2, D).astype(np.float32)

    x_i = ns[ei[:, 0]]
    msg = np.einsum("ed,edf->ef", x_i, a)
    summed = np.zeros((n_nodes, D), np.float32)
    np.add.at(summed, ei[:, 1], msg)
    cnt = np.bincount(ei[:, 1], minlength=n_nodes).astype(np.float32)
    expected = np.maximum(summed / np.maximum(cnt, 1.0)[:, None] + b[None, :], 0.0)

    if os.environ.get("RUN_HW"):
        actual = kernel(ns, ei, a, b)
    else:
        from concourse.bass_interp import CoreSim

        in_maps, meta = _prep(ns, ei, a, b)
        nc = _build(meta["TS"], enable_asserts=True)
        outs = []
        for c in range(NCORES):
            sim = CoreSim(nc, trace=False)
            for k, v in in_maps[c].items():
                sim.tensor(k)[:] = v
            sim.simulate()
            outs.append({"out": np.array(sim.tensor("out"))})
        actual = _finalize(outs, meta)

    err = np.abs(actual - expected)
    denom = np.abs(expected).max()
    print("max abs err:", err.max(), "rel to scale:", err.max() / denom)
    rel = np.linalg.norm(actual - expected) / np.linalg.norm(expected)
    print("l2 rel:", rel)
    assert rel < 2e-2, "FAIL"
    print("PASS")


# revision 16
# speedup vs baseline: 35.0697x; 1.2821x over previous
"""Edge-parallel GNN message passing on 8 Trainium2 NeuronCores.

Strategy (host-permuted, fully core-independent, bf16 stream):
  * Sort edges by destination node. Pack whole destination segments into
    128-edge tiles (max 32 segments per tile, so segment sums fit the 32
    output partitions of one PSUM tile). Tiles are dealt contiguously to
    the 8 cores -> no collective needed.
  * Per 128-edge tile, on device:
      stage 1: 32 bf16 matmuls, each computing 4 edges' (x_src @ A_e) via
               a block-diagonal x operand (K=128 = 4 edges x 32 dims):
               msgT[32f, 4e] = A_block[128,32].T-contract x_block[128,4].
      DVE stream-transpose flips each 32x32 block of msgT [32,128] so
               chunk b holds msg rows for edges 32b..32b+32 on
               partitions 0-31.
      stage 2: 4 accumulating K=32 matmuls with a recip-weighted one-hot
               selector S'[e',m] = (slot(e')==m)/count built on-device
               (DVE tensor_scalar is_equal*mult against an iota tile),
               yielding the segment MEAN directly; plus one K=1 rank-1
               matmul ones[1,32] x bias[1,32] that adds bias to every
               slot row. Epilogue is a single ACT Relu.
  * A/x/metadata are streamed as ONE fused bf16 DMA per 8-tile
    super-tile (the 2 GB a_in stream dominates; bf16 halves it).
  * Host scatters the per-(tile,slot) rows to node ids; isolated nodes
    get relu(bias); in-degree > 128 nodes fall back to host compute.
"""

import math
import os
from contextlib import ExitStack

import numpy as np
import ml_dtypes

import concourse.bass as bass  # noqa: F401
import concourse.tile as tile
from concourse import bacc, mybir
from concourse.bass_utils import run_bass_kernel_spmd

F32 = mybir.dt.float32
BF16 = mybir.dt.bfloat16
NPBF16 = ml_dtypes.bfloat16
NCORES = 8
D = 32
EPT = 128          # edges per tile
GPT = EPT // 4     # stage-1 matmul groups per tile
NSLOT = 32         # max destination segments per tile
SB = 8             # edge-tiles per super-tile (one fused DMA each)
ACOLS = GPT * D    # 1024
XCOLS = GPT        # 32
TCOLS = ACOLS + XCOLS  # 1056 bf16 columns per tile
MCOLS = 8          # rank4 | recip4 (separate f32 stream)
PAD_RANK = -1.0e9


def _pack_segments(counts):
    """Greedy-pack whole segments (each <= EPT edges) into tiles holding
    at most EPT edges and NSLOT segments."""
    n = len(counts)
    tile_id = np.empty(n, np.int64)
    slot = np.empty(n, np.int64)
    t = 0
    used = 0
    nseg = 0
    for i in range(n):
        c = counts[i]
        if used + c > EPT or nseg == NSLOT:
            t += 1
            used = 0
            nseg = 0
        tile_id[i] = t
        slot[i] = nseg
        used += c
        nseg += 1
    return tile_id, slot, (t + 1 if n else 0)


def _prep(node_states, edge_index, a_in, bias):
    ns = np.asarray(node_states, dtype=np.float32)
    ei = np.asarray(edge_index)
    a = np.asarray(a_in, dtype=np.float32)
    b = np.asarray(bias, dtype=np.float32)
    n_nodes, d = ns.shape
    assert d == D
    src = np.ascontiguousarray(ei[:, 0]).astype(np.int64)
    dst = np.ascontiguousarray(ei[:, 1]).astype(np.int64)

    perm = np.argsort(dst, kind="stable")
    dsts = dst[perm]
    nodes_u, counts = np.unique(dsts, return_counts=True)

    # Oversize segments (in-degree > EPT) fall back to host compute.
    big = counts > EPT
    host_nodes = nodes_u[big]
    edge_big = np.repeat(big, counts)
    perm_k = perm[~edge_big]
    nodes_k = nodes_u[~big]
    counts_k = counts[~big]

    tile_id, slot, n_tiles = _pack_segments(counts_k)
    n_tiles = max(n_tiles, 1)
    TS = int(math.ceil(n_tiles / (NCORES * SB)))   # super-tiles per core
    T = TS * SB                                    # edge-tiles per core
    Ttot = T * NCORES

    ek = len(perm_k)
    if ek:
        e_tile = np.repeat(tile_id, counts_k)
        cum_excl = np.concatenate(([0], np.cumsum(counts_k)))[:-1]
        tile_first_seg = np.searchsorted(tile_id, np.arange(n_tiles))
        tile_edge_start = cum_excl[tile_first_seg]
        e_pos = np.arange(ek) - tile_edge_start[e_tile]
        flat = e_tile * EPT + e_pos
    else:
        flat = np.zeros(0, np.int64)

    ei_flat = np.zeros(Ttot * EPT, np.int64)
    rank_flat = np.full(Ttot * EPT, PAD_RANK, np.float32)
    recip_flat = np.zeros(Ttot * EPT, np.float32)
    if ek:
        ei_flat[flat] = perm_k
        rank_flat[flat] = np.repeat(slot, counts_k).astype(np.float32)
        recip_flat[flat] = np.repeat(
            (1.0 / counts_k).astype(np.float32), counts_k
        )
    flatslot = tile_id * NSLOT + slot

    # One fused bf16 device stream per super-tile: [128, SB, TCOLS] where
    # per tile s the columns are
    #   [0,    1024)  A:  [p=32j+d, 32g+f] = a[e(4g+j), d, f]
    #   [1024, 1056)  x:  [p=32j+d, g]     = ns[src(e(4g+j)), d]
    # plus a small f32 metadata stream [32, SB, 8] per super-tile:
    #   [., s, b]     rank4  [e'<32]       = slot(e=32b+e') or -1e9 pad
    #   [., s, 4+b]   recip4 [e'<32]       = 1/count(segment of e) or 0
    AXR_host = np.zeros((NCORES, TS, 128, SB, TCOLS), NPBF16)
    META_host = np.zeros((NCORES, TS, 32, SB, MCOLS), np.float32)
    ei_r = ei_flat.reshape(NCORES, T * EPT)
    xsrc = src[ei_flat].reshape(NCORES, T * EPT)
    rank_r = rank_flat.reshape(NCORES, TS, SB, 4, 32)
    recip_r = recip_flat.reshape(NCORES, TS, SB, 4, 32)
    for c in range(NCORES):
        ae = a[ei_r[c]]                                   # [T*EPT, D, D]
        AXR_host[c, :, :, :, :ACOLS] = (
            ae.reshape(TS, SB, GPT, 4, D, D)
            .transpose(0, 3, 4, 1, 2, 5)                  # [t', j, d, s, g, f]
            .reshape(TS, 128, SB, ACOLS)
        )
        del ae
        xg = ns[xsrc[c]]                                  # [T*EPT, D]
        AXR_host[c, :, :, :, ACOLS:ACOLS + XCOLS] = (
            xg.reshape(TS, SB, GPT, 4, D)
            .transpose(0, 3, 4, 1, 2)                     # [t', j, d, s, g]
            .reshape(TS, 128, SB, XCOLS)
        )
        del xg
        META_host[c, :, :, :, :4] = rank_r[c].transpose(0, 3, 1, 2)
        META_host[c, :, :, :, 4:] = recip_r[c].transpose(0, 3, 1, 2)

    iota_host = np.tile(np.arange(NSLOT, dtype=np.float32), (32, 1))
    ob_host = np.zeros((1, 64), NPBF16)
    ob_host[0, :32] = 1.0
    ob_host[0, 32:] = b

    in_maps = [
        {
            "axr": AXR_host[c].reshape(TS, 128, SB * TCOLS),
            "meta": META_host[c].reshape(TS, 32, SB * MCOLS),
            "iota": iota_host,
            "onesbias": ob_host,
        }
        for c in range(NCORES)
    ]

    host_rows = None
    if len(host_nodes):
        eb = perm[edge_big]
        msg = np.einsum("ed,edf->ef", ns[src[eb]], a[eb])
        summed = np.zeros((len(host_nodes), D), np.float32)
        hn_index = {n: i for i, n in enumerate(host_nodes)}
        idx = np.fromiter((hn_index[n] for n in dst[eb]), np.int64, len(eb))
        np.add.at(summed, idx, msg)
        cnt = counts[big].astype(np.float32)[:, None]
        host_rows = np.maximum(summed / cnt + b[None, :], 0.0).astype(np.float32)

    meta = dict(
        n_nodes=n_nodes,
        TS=TS,
        nodes_k=nodes_k,
        flatslot=flatslot,
        host_nodes=host_nodes,
        host_rows=host_rows,
        bias=b,
    )
    return in_maps, meta


def _build(TS, enable_asserts=False, repeat=1):
    nc = bacc.Bacc(
        "TRN2",
        target_bir_lowering=False,
        debug=False,
        enable_asserts=enable_asserts,
        num_devices=NCORES,
    )
    axr_d = nc.dram_tensor("axr", [TS, 128, SB * TCOLS], BF16, kind="ExternalInput")
    meta_d = nc.dram_tensor("meta", [TS, 32, SB * MCOLS], F32, kind="ExternalInput")
    iota_d = nc.dram_tensor("iota", [32, NSLOT], F32, kind="ExternalInput")
    ob_d = nc.dram_tensor("onesbias", [1, 64], BF16, kind="ExternalInput")
    out_d = nc.dram_tensor("out", [TS, NSLOT, SB * D], F32, kind="ExternalOutput")

    with tile.TileContext(nc) as tc, ExitStack() as ctx:
        cpool = ctx.enter_context(tc.tile_pool(name="const", bufs=1))
        apool = ctx.enter_context(tc.tile_pool(name="apool", bufs=3))
        spool = ctx.enter_context(tc.tile_pool(name="spool", bufs=3))
        wpool = ctx.enter_context(tc.tile_pool(name="wpool", bufs=4))
        opool = ctx.enter_context(tc.tile_pool(name="opool", bufs=2))
        ps_m = ctx.enter_context(tc.tile_pool(name="ps_m", bufs=2, space="PSUM"))
        ps_s = ctx.enter_context(tc.tile_pool(name="ps_s", bufs=2, space="PSUM"))

        iota_t = cpool.tile([32, NSLOT], F32, tag="iota")
        nc.sync.dma_start(iota_t[:], iota_d[:])
        ob_t = cpool.tile([1, 64], BF16, tag="ob")
        nc.sync.dma_start(ob_t[:], ob_d[:])

        # Two persistent block-diagonal x operands (one per super parity);
        # off-diagonal cells zeroed once (spread copies only touch the
        # diagonal 32-row blocks, so reuse keeps them zero).
        xm = []
        for i in range(2):
            t_ = cpool.tile([128, SB, 4, GPT], BF16, tag=f"xmega{i}")
            nc.vector.memset(t_[:], 0.0)
            xm.append(t_)

        def dma_in(q):
            t_ = apool.tile([128, SB * TCOLS], BF16, tag="axr")
            nc.sync.dma_start(t_[:], axr_d[q])
            m_ = spool.tile([32, SB * MCOLS], F32, tag="meta")
            nc.sync.dma_start(m_[:], meta_d[q])
            return t_, m_

        # repeat>1 unrolls the whole body again over the same inputs — a
        # timing-only variant so per-invocation device time can be read off
        # the slope of repeated-execute wall time (outputs are rewritten
        # with identical values each rep).
        qs = [q for _ in range(repeat) for q in range(TS)]
        at_buf = {0: dma_in(qs[0])}
        if len(qs) > 1:
            at_buf[1] = dma_in(qs[1])

        pend = None  # (s_t, msgTT, osup, s) of the previous tile

        def finish(pend_v):
            s_t, msgTT, osup_, s_ = pend_v
            sum_ps = ps_s.tile([NSLOT, D], F32, tag="sum")
            for bk in range(4):
                nc.tensor.matmul(
                    sum_ps[:],
                    s_t[:, bk, :],
                    msgTT[:, 32 * bk : 32 * bk + 32],
                    start=(bk == 0),
                    stop=False,
                )
            nc.tensor.matmul(
                sum_ps[:], ob_t[:, 0:32], ob_t[:, 32:64], start=False, stop=True
            )
            nc.scalar.activation(
                osup_[:, D * s_ : D * s_ + D],
                sum_ps[:],
                mybir.ActivationFunctionType.Relu,
            )

        osup = None
        for pos, q in enumerate(qs):
            at, mt = at_buf.pop(pos)
            if pos + 2 < len(qs):
                at_buf[pos + 2] = dma_in(qs[pos + 2])
            atv = at.rearrange("p (s c) -> p s c", s=SB)
            xv = xm[pos % 2]

            # Spread compact x columns into the block-diagonal operand
            # (column-only moves within each 32-partition slab; split
            # across DVE and ACT to balance engine load).
            for j in range(4):
                dst_ = xv[32 * j : 32 * j + 32, :, j, :]
                src_ = atv[32 * j : 32 * j + 32, :, ACOLS : ACOLS + XCOLS]
                if j < 2:
                    nc.vector.tensor_copy(dst_, src_)
                else:
                    nc.scalar.copy(dst_, src_)

            osup_prev = osup
            osup = opool.tile([NSLOT, SB * D], F32, tag="osup")

            for s in range(SB):
                # stage 1: msgT[32f, 128e] in PSUM
                msgT_ps = ps_m.tile([32, EPT], F32, tag="msgT")
                for g in range(GPT):
                    nc.tensor.matmul(
                        msgT_ps[:, 4 * g : 4 * g + 4],
                        atv[:, s, 32 * g : 32 * g + 32],
                        xv[:, s, :, g],
                        start=True,
                        stop=True,
                    )
                # selector S'[e', b, m] = (slot==m) * recip  (bf16)
                s_t = spool.tile([32, 4, NSLOT], BF16, tag="S")
                for bk in range(4):
                    nc.vector.tensor_scalar(
                        s_t[:, bk, :],
                        iota_t[:],
                        mt[:, MCOLS * s + bk : MCOLS * s + bk + 1],
                        mt[:, MCOLS * s + 4 + bk : MCOLS * s + 5 + bk],
                        mybir.AluOpType.is_equal,
                        mybir.AluOpType.mult,
                    )
                # PSUM -> SBUF (bf16) then 32x32 block transpose on DVE
                msgT_sb = wpool.tile([32, EPT], BF16, tag="msgTsb")
                nc.scalar.copy(msgT_sb[:], msgT_ps[:])
                msgTT = wpool.tile([32, EPT], BF16, tag="msgTT")
                nc.vector.transpose(msgTT[:], msgT_sb[:])

                if pend is not None:
                    finish(pend)
                    if s == 0 and pos > 0:
                        nc.sync.dma_start(out_d[qs[pos - 1]], osup_prev[:])
                pend = (s_t, msgTT, osup, s)

        finish(pend)
        nc.sync.dma_start(out_d[qs[-1]], osup[:])

    nc.compile()
    return nc


_BUILD_CACHE = {}


def _built(TS):
    nc = _BUILD_CACHE.get(TS)
    if nc is None:
        nc = _build(TS)
        _BUILD_CACHE[TS] = nc
    return nc


def _finalize(results, meta):
    sup = np.concatenate([r["out"] for r in results], axis=0)  # [NC*TS,32,SB*D]
    ncts = sup.shape[0]
    rows = (
        sup.reshape(ncts, NSLOT, SB, D)
        .transpose(0, 2, 1, 3)                                 # [t', s, m, f]
        .reshape(-1, D)
    )
    b = meta["bias"]
    out = np.empty((meta["n_nodes"], D), np.float32)
    out[:] = np.maximum(b, 0.0)[None, :]
    out[meta["nodes_k"]] = rows[meta["flatslot"]]
    if meta["host_rows"] is not None:
        out[meta["host_nodes"]] = meta["host_rows"]
    return out


def kernel(node_states, edge_index, a_in, bias):
    in_maps, meta = _prep(node_states, edge_index, a_in, bias)
    nc = _built(meta["TS"])
    res = run_bass_kernel_spmd(nc, in_maps, list(range(NCORES)))
    return _finalize(res.results, meta)


if __name__ == "__main__":
    np.random.seed(0)
    n_nodes, n_edges = 700, 3000
    ns = np.random.randn(n_nodes, D).astype(np.float32)
    ei = np.random.randint(0, n_nodes, (n_edges, 2)).astype(np.int64)
    a = (np.random.randn(n_edges, D, D) / np.sqrt(D)).astype(np.float32)
    b = np.random.uniform(-0.2, 0.2, D).astype(np.float32)

    x_i = ns[ei[:, 0]]
    msg = np.einsum("ed,edf->ef", x_i, a)
    summed = np.zeros((n_nodes, D), np.float32)
    np.add.at(summed, ei[:, 1], msg)
    cnt = np.bincount(ei[:, 1], minlength=n_nodes).astype(np.float32)
    expected = np.maximum(summed / np.maximum(cnt, 1.0)[:, None] + b[None, :], 0.0)

    if os.environ.get("RUN_HW"):
        actual = kernel(ns, ei, a, b)
    else:
        from concourse.bass_interp import CoreSim

        in_maps, meta = _prep(ns, ei, a, b)
        nc = _build(meta["TS"], enable_asserts=True)
        outs = []
        for c in range(NCORES):
            sim = CoreSim(nc, trace=False)
            for k, v in in_maps[c].items():
                sim.tensor(k)[:] = v
            sim.simulate()
            outs.append({"out": np.array(sim.tensor("out"))})
        actual = _finalize(outs, meta)

    err = np.abs(actual - expected)
    denom = np.abs(expected).max()
    print("max abs err:", err.max(), "rel to scale:", err.max() / denom)
    rel = np.linalg.norm(actual - expected) / np.linalg.norm(expected)
    print("l2 rel:", rel)
    assert rel < 2e-2, "FAIL"
    print("PASS")


# revision 24
# speedup vs baseline: 35.5717x; 1.0143x over previous
"""Edge-parallel GNN message passing on 8 Trainium2 NeuronCores.

Strategy (host-permuted, fully core-independent, bf16 stream):
  * Sort edges by destination node. Pack whole destination segments into
    128-edge tiles (max 32 segments per tile, so segment sums fit the 32
    output partitions of one PSUM tile). Tiles are dealt contiguously to
    the 8 cores -> no collective needed.
  * Per 128-edge tile, on device:
      stage 1: 32 bf16 matmuls, each computing 4 edges' (x_src @ A_e) via
               a block-diagonal x operand (K=128 = 4 edges x 32 dims):
               msgT[32f, 4e] = A_block[128,32].T-contract x_block[128,4].
      DVE stream-transpose flips each 32x32 block of msgT [32,128] so
               chunk b holds msg rows for edges 32b..32b+32 on
               partitions 0-31.
      stage 2: 4 accumulating K=32 matmuls with a recip-weighted one-hot
               selector S'[e',m] = (slot(e')==m)/count built on-device
               (DVE tensor_scalar is_equal*mult against an iota tile),
               yielding the segment MEAN directly; plus one K=1 rank-1
               matmul ones[1,32] x bias[1,32] that adds bias to every
               slot row. Epilogue is a single ACT Relu.
  * A/x/metadata are streamed as ONE fused bf16 DMA per 8-tile
    super-tile (the 2 GB a_in stream dominates; bf16 halves it).
  * Host scatters the per-(tile,slot) rows to node ids; isolated nodes
    get relu(bias); in-degree > 128 nodes fall back to host compute.
"""

import math
import os
from contextlib import ExitStack

import numpy as np
import ml_dtypes

import concourse.bass as bass  # noqa: F401
import concourse.tile as tile
from concourse import bacc, mybir
from concourse.bass_utils import run_bass_kernel_spmd

F32 = mybir.dt.float32
BF16 = mybir.dt.bfloat16
NPBF16 = ml_dtypes.bfloat16
NCORES = 8
D = 32
EPT = 128          # edges per tile
GPT = EPT // 4     # stage-1 matmul groups per tile
NSLOT = 32         # max destination segments per tile
SB = 8             # edge-tiles per super-tile (one fused DMA each)
ACOLS = GPT * D    # 1024
XCOLS = GPT        # 32
TCOLS = ACOLS + XCOLS  # 1056 bf16 columns per tile
MCOLS = 8          # rank4 | recip4 (separate f32 stream)
PAD_RANK = -1.0e9


def _pack_segments(counts):
    """Greedy-pack whole segments (each <= EPT edges) into tiles holding
    at most EPT edges and NSLOT segments."""
    n = len(counts)
    tile_id = np.empty(n, np.int64)
    slot = np.empty(n, np.int64)
    t = 0
    used = 0
    nseg = 0
    for i in range(n):
        c = counts[i]
        if used + c > EPT or nseg == NSLOT:
            t += 1
            used = 0
            nseg = 0
        tile_id[i] = t
        slot[i] = nseg
        used += c
        nseg += 1
    return tile_id, slot, (t + 1 if n else 0)


def _prep(node_states, edge_index, a_in, bias):
    ns = np.asarray(node_states, dtype=np.float32)
    ei = np.asarray(edge_index)
    a = np.asarray(a_in, dtype=np.float32)
    b = np.asarray(bias, dtype=np.float32)
    n_nodes, d = ns.shape
    assert d == D
    src = np.ascontiguousarray(ei[:, 0]).astype(np.int64)
    dst = np.ascontiguousarray(ei[:, 1]).astype(np.int64)

    perm = np.argsort(dst, kind="stable")
    dsts = dst[perm]
    nodes_u, counts = np.unique(dsts, return_counts=True)

    # Oversize segments (in-degree > EPT) fall back to host compute.
    big = counts > EPT
    host_nodes = nodes_u[big]
    edge_big = np.repeat(big, counts)
    perm_k = perm[~edge_big]
    nodes_k = nodes_u[~big]
    counts_k = counts[~big]

    tile_id, slot, n_tiles = _pack_segments(counts_k)
    n_tiles = max(n_tiles, 1)
    TS = int(math.ceil(n_tiles / (NCORES * SB)))   # super-tiles per core
    T = TS * SB                                    # edge-tiles per core
    Ttot = T * NCORES

    ek = len(perm_k)
    if ek:
        e_tile = np.repeat(tile_id, counts_k)
        cum_excl = np.concatenate(([0], np.cumsum(counts_k)))[:-1]
        tile_first_seg = np.searchsorted(tile_id, np.arange(n_tiles))
        tile_edge_start = cum_excl[tile_first_seg]
        e_pos = np.arange(ek) - tile_edge_start[e_tile]
        flat = e_tile * EPT + e_pos
    else:
        flat = np.zeros(0, np.int64)

    ei_flat = np.zeros(Ttot * EPT, np.int64)
    rank_flat = np.full(Ttot * EPT, PAD_RANK, np.float32)
    recip_flat = np.zeros(Ttot * EPT, np.float32)
    if ek:
        ei_flat[flat] = perm_k
        rank_flat[flat] = np.repeat(slot, counts_k).astype(np.float32)
        recip_flat[flat] = np.repeat(
            (1.0 / counts_k).astype(np.float32), counts_k
        )
    flatslot = tile_id * NSLOT + slot

    # One fused bf16 device stream per super-tile: [128, SB, TCOLS] where
    # per tile s the columns are
    #   [0,    1024)  A:  [p=32j+d, 32g+f] = a[e(4g+j), d, f]
    #                 (quad k's [128,128] stationary = cols 128k..128k+128)
    #   [1024, 1056)  x:  [p=32j+d, g]     = ns[src(e(4g+j)), d]
    # plus an f32 metadata stream [128, SB, 8] per super-tile holding the
    # rank/recip of each edge at the SBUF row where its message lands after
    # the diagonal-block matmul + 32x32 stream-transpose:
    #   row r = 33*gj + 16*(k%2) + 4*j, chunk cb = k//2
    #   for edge pos = 16k + 4*(4?…)  (pos: g = pos//4 = 4k+gj, j = pos%4)
    #   [., s, cb]    rank   or -1e9 pad;  [., s, 4+cb]  recip or 0
    AXR_host = np.zeros((NCORES, TS, 128, SB, TCOLS), NPBF16)
    ei_r = ei_flat.reshape(NCORES, T * EPT)
    xsrc = src[ei_flat].reshape(NCORES, T * EPT)
    for c in range(NCORES):
        ae = a[ei_r[c]]                                   # [T*EPT, D, D]
        AXR_host[c, :, :, :, :ACOLS] = (
            ae.reshape(TS, SB, GPT, 4, D, D)
            .transpose(0, 3, 4, 1, 2, 5)                  # [t', j, d, s, g, f]
            .reshape(TS, 128, SB, ACOLS)
        )
        del ae
        xg = ns[xsrc[c]]                                  # [T*EPT, D]
        AXR_host[c, :, :, :, ACOLS:ACOLS + XCOLS] = (
            xg.reshape(TS, SB, GPT, 4, D)
            .transpose(0, 3, 4, 1, 2)                     # [t', j, d, s, g]
            .reshape(TS, 128, SB, XCOLS)
        )
        del xg

    meta_flat = np.zeros((Ttot, 128, MCOLS), np.float32)
    meta_flat[:, :, :4] = PAD_RANK
    if ek:
        gq = (flat % EPT) // 4                   # group within tile
        jq = flat % 4
        kq = gq // 4                             # quad
        gjq = gq % 4
        rq = 33 * gjq + 16 * (kq % 2) + 4 * jq   # post-transpose SBUF row
        cbq = kq // 2                            # 32-col chunk
        tq = flat // EPT
        meta_flat[tq, rq, cbq] = np.repeat(slot, counts_k).astype(np.float32)
        meta_flat[tq, rq, 4 + cbq] = np.repeat(
            (1.0 / counts_k).astype(np.float32), counts_k
        )
    META_host = (
        meta_flat.reshape(NCORES, TS, SB, 128, MCOLS)
        .transpose(0, 1, 3, 2, 4)                # [c, t', p, s, mcol]
        .copy()
    )

    iota_host = np.tile(np.arange(NSLOT, dtype=np.float32), (128, 1))
    ob_host = np.zeros((1, 64), NPBF16)
    ob_host[0, :32] = 1.0
    ob_host[0, 32:] = b

    in_maps = [
        {
            "axr": AXR_host[c].reshape(TS, 128, SB * TCOLS),
            "meta": META_host[c].reshape(TS, 128, SB * MCOLS),
            "iota": iota_host,
            "onesbias": ob_host,
        }
        for c in range(NCORES)
    ]

    host_rows = None
    if len(host_nodes):
        eb = perm[edge_big]
        msg = np.einsum("ed,edf->ef", ns[src[eb]], a[eb])
        summed = np.zeros((len(host_nodes), D), np.float32)
        hn_index = {n: i for i, n in enumerate(host_nodes)}
        idx = np.fromiter((hn_index[n] for n in dst[eb]), np.int64, len(eb))
        np.add.at(summed, idx, msg)
        cnt = counts[big].astype(np.float32)[:, None]
        host_rows = np.maximum(summed / cnt + b[None, :], 0.0).astype(np.float32)

    meta = dict(
        n_nodes=n_nodes,
        TS=TS,
        nodes_k=nodes_k,
        flatslot=flatslot,
        host_nodes=host_nodes,
        host_rows=host_rows,
        bias=b,
    )
    return in_maps, meta


def _build(TS, enable_asserts=False, repeat=1):
    nc = bacc.Bacc(
        "TRN2",
        target_bir_lowering=False,
        debug=False,
        enable_asserts=enable_asserts,
        num_devices=NCORES,
    )
    axr_d = nc.dram_tensor("axr", [TS, 128, SB * TCOLS], BF16, kind="ExternalInput")
    meta_d = nc.dram_tensor("meta", [TS, 128, SB * MCOLS], F32, kind="ExternalInput")
    iota_d = nc.dram_tensor("iota", [128, NSLOT], F32, kind="ExternalInput")
    ob_d = nc.dram_tensor("onesbias", [1, 64], BF16, kind="ExternalInput")
    out_d = nc.dram_tensor("out", [TS, NSLOT, SB * D], F32, kind="ExternalOutput")

    with tile.TileContext(nc) as tc, ExitStack() as ctx:
        cpool = ctx.enter_context(tc.tile_pool(name="const", bufs=1))
        apool = ctx.enter_context(tc.tile_pool(name="apool", bufs=3))
        spool = ctx.enter_context(tc.tile_pool(name="spool", bufs=3))
        wpool = ctx.enter_context(tc.tile_pool(name="wpool", bufs=4))
        opool = ctx.enter_context(tc.tile_pool(name="opool", bufs=2))
        ps_m = ctx.enter_context(tc.tile_pool(name="ps_m", bufs=2, space="PSUM"))
        ps_s = ctx.enter_context(tc.tile_pool(name="ps_s", bufs=2, space="PSUM"))

        iota_t = cpool.tile([128, NSLOT], F32, tag="iota")
        nc.sync.dma_start(iota_t[:], iota_d[:])
        ob_t = cpool.tile([1, 64], BF16, tag="ob")
        nc.sync.dma_start(ob_t[:], ob_d[:])

        # Two persistent block-diagonal x operands (one per super parity);
        # off-diagonal cells zeroed once (spread copies only touch the
        # diagonal 32-row blocks, so reuse keeps them zero).
        xm = []
        for i in range(2):
            t_ = cpool.tile([128, SB, 4, GPT], BF16, tag=f"xmega{i}")
            nc.vector.memset(t_[:], 0.0)
            xm.append(t_)

        def dma_in(q):
            t_ = apool.tile([128, SB * TCOLS], BF16, tag="axr")
            nc.sync.dma_start(t_[:], axr_d[q])
            m_ = spool.tile([128, SB * MCOLS], F32, tag="meta")
            nc.sync.dma_start(m_[:], meta_d[q])
            return t_, m_

        # repeat>1 unrolls the whole body again over the same inputs — a
        # timing-only variant so per-invocation device time can be read off
        # the slope of repeated-execute wall time (outputs are rewritten
        # with identical values each rep).
        qs = [q for _ in range(repeat) for q in range(TS)]
        at_buf = {0: dma_in(qs[0])}
        if len(qs) > 1:
            at_buf[1] = dma_in(qs[1])

        pend = None  # (s_t, msgTT, osup, s) of the previous tile

        def finish(pend_v):
            s_t, msgTT, osup_, s_ = pend_v
            sum_ps = ps_s.tile([NSLOT, D], F32, tag="sum")
            for cb in range(4):
                nc.tensor.matmul(
                    sum_ps[:],
                    s_t[:, cb, :],
                    msgTT[:, 32 * cb : 32 * cb + 32],
                    start=(cb == 0),
                    stop=False,
                )
            nc.tensor.matmul(
                sum_ps[:], ob_t[:, 0:32], ob_t[:, 32:64], start=False, stop=True
            )
            nc.scalar.activation(
                osup_[:, D * s_ : D * s_ + D],
                sum_ps[:],
                mybir.ActivationFunctionType.Relu,
            )

        osup = None
        for pos, q in enumerate(qs):
            at, mt = at_buf.pop(pos)
            if pos + 2 < len(qs):
                at_buf[pos + 2] = dma_in(qs[pos + 2])
            atv = at.rearrange("p (s c) -> p s c", s=SB)
            xv = xm[pos % 2]

            # Spread compact x columns into the block-diagonal operand
            # (column-only moves within each 32-partition slab; split
            # across DVE and ACT to balance engine load).
            for j in range(4):
                dst_ = xv[32 * j : 32 * j + 32, :, j, :]
                src_ = atv[32 * j : 32 * j + 32, :, ACOLS : ACOLS + XCOLS]
                if j < 2:
                    nc.vector.tensor_copy(dst_, src_)
                else:
                    nc.scalar.copy(dst_, src_)

            osup_prev = osup
            osup = opool.tile([NSLOT, SB * D], F32, tag="osup")

            for s in range(SB):
                # stage 1: 8 quad matmuls. lhsT = A4 [128,128] (4 groups'
                # A blocks; 128-col bf16 stationary -> compiler enables FWL,
                # 2x weight-load) x block-diag x [128,16] -> [128,16] whose
                # diagonal 32x4 blocks are the quad's msgT; garbage
                # elsewhere. The host-side rank permutation (row formula in
                # _prep) selects the useful rows after the block transpose.
                msgT_ps = ps_m.tile([128, 8, 16], F32, tag="msgT")
                for k in range(8):
                    nc.tensor.matmul(
                        msgT_ps[:, k, :],
                        atv[:, s, 128 * k : 128 * k + 128],
                        xv[:, s, :, 4 * k : 4 * k + 4],
                        start=True,
                        stop=True,
                    )
                # selector S'[r, cb, m] = (rank[r]==m) * recip[r]  (bf16);
                # split across DVE and GPSIMD to balance engine load
                s_t = spool.tile([128, 4, NSLOT], BF16, tag="S")
                for cb in range(4):
                    eng = nc.vector if cb < 2 else nc.gpsimd
                    eng.tensor_scalar(
                        s_t[:, cb, :],
                        iota_t[:],
                        mt[:, MCOLS * s + cb : MCOLS * s + cb + 1],
                        mt[:, MCOLS * s + 4 + cb : MCOLS * s + 5 + cb],
                        mybir.AluOpType.is_equal,
                        mybir.AluOpType.mult,
                    )
                # PSUM -> SBUF (bf16) then 32x32 block transpose on DVE
                msgT_sb = wpool.tile([128, EPT], BF16, tag="msgTsb")
                nc.scalar.copy(
                    msgT_sb[:], msgT_ps.rearrange("p a b -> p (a b)")
                )
                msgTT = wpool.tile([128, EPT], BF16, tag="msgTT")
                nc.vector.transpose(msgTT[:], msgT_sb[:])

                if pend is not None:
                    finish(pend)
                    if s == 0 and pos > 0:
                        nc.sync.dma_start(out_d[qs[pos - 1]], osup_prev[:])
                pend = (s_t, msgTT, osup, s)

        finish(pend)
        nc.sync.dma_start(out_d[qs[-1]], osup[:])

    nc.compile()
    return nc


_BUILD_CACHE = {}


def _built(TS):
    nc = _BUILD_CACHE.get(TS)
    if nc is None:
        nc = _build(TS)
        _BUILD_CACHE[TS] = nc
    return nc


def _finalize(results, meta):
    sup = np.concatenate([r["out"] for r in results], axis=0)  # [NC*TS,32,SB*D]
    ncts = sup.shape[0]
    rows = (
        sup.reshape(ncts, NSLOT, SB, D)
        .transpose(0, 2, 1, 3)                                 # [t', s, m, f]
        .reshape(-1, D)
    )
    b = meta["bias"]
    out = np.empty((meta["n_nodes"], D), np.float32)
    out[:] = np.maximum(b, 0.0)[None, :]
    out[meta["nodes_k"]] = rows[meta["flatslot"]]
    if meta["host_rows"] is not None:
        out[meta["host_nodes"]] = meta["host_rows"]
    return out


def kernel(node_states, edge_index, a_in, bias):
    in_maps, meta = _prep(node_states, edge_index, a_in, bias)
    nc = _built(meta["TS"])
    res = run_bass_kernel_spmd(nc, in_maps, list(range(NCORES)))
    return _finalize(res.results, meta)


if __name__ == "__main__":
    np.random.seed(0)
    n_nodes, n_edges = 700, 3000
    ns = np.random.randn(n_nodes, D).astype(np.float32)
    ei = np.random.randint(0, n_nodes, (n_edges, 2)).astype(np.int64)
    a = (np.random.randn(n_edges, D, D) / np.sqrt(D)).astype(np.float32)
    b = np.random.uniform(-0.2, 0.2, D).astype(np.float32)

    x_i = ns[ei[:, 0]]
    msg = np.einsum("ed,edf->ef", x_i, a)
    summed = np.zeros((n_nodes, D), np.float32)
    np.add.at(summed, ei[:, 1], msg)
    cnt = np.bincount(ei[:, 1], minlength=n_nodes).astype(np.float32)
    expected = np.maximum(summed / np.maximum(cnt, 1.0)[:, None] + b[None, :], 0.0)

    if os.environ.get("RUN_HW"):
        actual = kernel(ns, ei, a, b)
    else:
        from concourse.bass_interp import CoreSim

        in_maps, meta = _prep(ns, ei, a, b)
        nc = _build(meta["TS"], enable_asserts=True)
        outs = []
        for c in range(NCORES):
            sim = CoreSim(nc, trace=False)
            for k, v in in_maps[c].items():
                sim.tensor(k)[:] = v
            sim.simulate()
            outs.append({"out": np.array(sim.tensor("out"))})
        actual = _finalize(outs, meta)

    err = np.abs(actual - expected)
    denom = np.abs(expected).max()
    print("max abs err:", err.max(), "rel to scale:", err.max() / denom)
    rel = np.linalg.norm(actual - expected) / np.linalg.norm(expected)
    print("l2 rel:", rel)
    assert rel < 2e-2, "FAIL"
    print("PASS")


# revision 26
# speedup vs baseline: 38.1205x; 1.0717x over previous
"""Edge-parallel GNN message passing on 8 Trainium2 NeuronCores.

Strategy (host-permuted, fully core-independent, bf16 stream):
  * Sort edges by destination node. Pack whole destination segments into
    128-edge tiles (max 32 segments per tile, so segment sums fit the 32
    output partitions of one PSUM tile). Tiles are dealt contiguously to
    the 8 cores -> no collective needed.
  * Per 128-edge tile, on device:
      stage 1: 32 bf16 matmuls, each computing 4 edges' (x_src @ A_e) via
               a block-diagonal x operand (K=128 = 4 edges x 32 dims):
               msgT[32f, 4e] = A_block[128,32].T-contract x_block[128,4].
      DVE stream-transpose flips each 32x32 block of msgT [32,128] so
               chunk b holds msg rows for edges 32b..32b+32 on
               partitions 0-31.
      stage 2: 4 accumulating K=32 matmuls with a recip-weighted one-hot
               selector S'[e',m] = (slot(e')==m)/count built on-device
               (DVE tensor_scalar is_equal*mult against an iota tile),
               yielding the segment MEAN directly; plus one K=1 rank-1
               matmul ones[1,32] x bias[1,32] that adds bias to every
               slot row. Epilogue is a single ACT Relu.
  * A/x/metadata are streamed as ONE fused bf16 DMA per 8-tile
    super-tile (the 2 GB a_in stream dominates; bf16 halves it).
  * Host scatters the per-(tile,slot) rows to node ids; isolated nodes
    get relu(bias); in-degree > 128 nodes fall back to host compute.
"""

import math
import os
from contextlib import ExitStack

import numpy as np
import ml_dtypes

import concourse.bass as bass  # noqa: F401
import concourse.tile as tile
from concourse import bacc, mybir
from concourse.bass_utils import run_bass_kernel_spmd

F32 = mybir.dt.float32
BF16 = mybir.dt.bfloat16
NPBF16 = ml_dtypes.bfloat16
NCORES = 8
D = 32
EPT = 128          # edges per tile
GPT = EPT // 4     # stage-1 matmul groups per tile
NSLOT = 32         # max destination segments per tile
SB = 8             # edge-tiles per super-tile (one fused DMA each)
ACOLS = GPT * D    # 1024
XCOLS = GPT        # 32
TCOLS = ACOLS + XCOLS  # 1056 bf16 columns per tile
MCOLS = 8          # rank4 | recip4 (separate f32 stream)
PAD_RANK = -1.0e9


def _pack_segments(counts):
    """Greedy-pack whole segments (each <= EPT edges) into tiles holding
    at most EPT edges and NSLOT segments."""
    n = len(counts)
    tile_id = np.empty(n, np.int64)
    slot = np.empty(n, np.int64)
    t = 0
    used = 0
    nseg = 0
    for i in range(n):
        c = counts[i]
        if used + c > EPT or nseg == NSLOT:
            t += 1
            used = 0
            nseg = 0
        tile_id[i] = t
        slot[i] = nseg
        used += c
        nseg += 1
    return tile_id, slot, (t + 1 if n else 0)


def _prep(node_states, edge_index, a_in, bias):
    ns = np.asarray(node_states, dtype=np.float32)
    ei = np.asarray(edge_index)
    a = np.asarray(a_in, dtype=np.float32)
    b = np.asarray(bias, dtype=np.float32)
    n_nodes, d = ns.shape
    assert d == D
    src = np.ascontiguousarray(ei[:, 0]).astype(np.int64)
    dst = np.ascontiguousarray(ei[:, 1]).astype(np.int64)

    perm = np.argsort(dst, kind="stable")
    dsts = dst[perm]
    nodes_u, counts = np.unique(dsts, return_counts=True)

    # Oversize segments (in-degree > EPT) fall back to host compute.
    big = counts > EPT
    host_nodes = nodes_u[big]
    edge_big = np.repeat(big, counts)
    perm_k = perm[~edge_big]
    nodes_k = nodes_u[~big]
    counts_k = counts[~big]

    tile_id, slot, n_tiles = _pack_segments(counts_k)
    n_tiles = max(n_tiles, 1)
    TS = int(math.ceil(n_tiles / (NCORES * SB)))   # super-tiles per core
    T = TS * SB                                    # edge-tiles per core
    Ttot = T * NCORES

    ek = len(perm_k)
    if ek:
        e_tile = np.repeat(tile_id, counts_k)
        cum_excl = np.concatenate(([0], np.cumsum(counts_k)))[:-1]
        tile_first_seg = np.searchsorted(tile_id, np.arange(n_tiles))
        tile_edge_start = cum_excl[tile_first_seg]
        e_pos = np.arange(ek) - tile_edge_start[e_tile]
        flat = e_tile * EPT + e_pos
    else:
        flat = np.zeros(0, np.int64)

    ei_flat = np.zeros(Ttot * EPT, np.int64)
    rank_flat = np.full(Ttot * EPT, PAD_RANK, np.float32)
    recip_flat = np.zeros(Ttot * EPT, np.float32)
    if ek:
        ei_flat[flat] = perm_k
        rank_flat[flat] = np.repeat(slot, counts_k).astype(np.float32)
        recip_flat[flat] = np.repeat(
            (1.0 / counts_k).astype(np.float32), counts_k
        )
    flatslot = tile_id * NSLOT + slot

    # One fused bf16 device stream per super-tile: [128, SB, TCOLS] where
    # per tile s the columns are
    #   [0,    1024)  A:  [p=32j+d, 32g+f] = a[e(4g+j), d, f]
    #                 (quad k's [128,128] stationary = cols 128k..128k+128)
    #   [1024, 1056)  x:  [p=32j+d, g]     = ns[src(e(4g+j)), d]
    # plus an f32 metadata stream [128, SB, 8] per super-tile holding the
    # rank/recip of each edge at the SBUF row where its message lands after
    # the diagonal-block matmul + 32x32 stream-transpose:
    #   row r = 33*gj + 16*(k%2) + 4*j, chunk cb = k//2
    #   for edge pos = 16k + 4*(4?…)  (pos: g = pos//4 = 4k+gj, j = pos%4)
    #   [., s, cb]    rank   or -1e9 pad;  [., s, 4+cb]  recip or 0
    AXR_host = np.zeros((NCORES, TS, 128, SB, TCOLS), NPBF16)
    ei_r = ei_flat.reshape(NCORES, T * EPT)
    xsrc = src[ei_flat].reshape(NCORES, T * EPT)
    for c in range(NCORES):
        ae = a[ei_r[c]]                                   # [T*EPT, D, D]
        AXR_host[c, :, :, :, :ACOLS] = (
            ae.reshape(TS, SB, GPT, 4, D, D)
            .transpose(0, 3, 4, 1, 2, 5)                  # [t', j, d, s, g, f]
            .reshape(TS, 128, SB, ACOLS)
        )
        del ae
        xg = ns[xsrc[c]]                                  # [T*EPT, D]
        AXR_host[c, :, :, :, ACOLS:ACOLS + XCOLS] = (
            xg.reshape(TS, SB, GPT, 4, D)
            .transpose(0, 3, 4, 1, 2)                     # [t', j, d, s, g]
            .reshape(TS, 128, SB, XCOLS)
        )
        del xg

    meta_flat = np.zeros((Ttot, 128, MCOLS), np.float32)
    meta_flat[:, :, :4] = PAD_RANK
    if ek:
        gq = (flat % EPT) // 4                   # group within tile
        jq = flat % 4
        kq = gq // 4                             # quad
        gjq = gq % 4
        rq = 33 * gjq + 16 * (kq % 2) + 4 * jq   # post-transpose SBUF row
        cbq = kq // 2                            # 32-col chunk
        tq = flat // EPT
        meta_flat[tq, rq, cbq] = np.repeat(slot, counts_k).astype(np.float32)
        meta_flat[tq, rq, 4 + cbq] = np.repeat(
            (1.0 / counts_k).astype(np.float32), counts_k
        )
    META_host = (
        meta_flat.reshape(NCORES, TS, SB, 128, MCOLS)
        .transpose(0, 1, 3, 2, 4)                # [c, t', p, s, mcol]
        .copy()
    )

    iota_host = np.tile(np.arange(NSLOT, dtype=np.float32), (128, 1))
    ob_host = np.zeros((1, 64), NPBF16)
    ob_host[0, :32] = 1.0
    ob_host[0, 32:] = b

    in_maps = [
        {
            "axr": AXR_host[c].reshape(TS, 128, SB * TCOLS),
            "meta": META_host[c].reshape(TS, 128, SB * MCOLS),
            "iota": iota_host,
            "onesbias": ob_host,
        }
        for c in range(NCORES)
    ]

    host_rows = None
    if len(host_nodes):
        eb = perm[edge_big]
        msg = np.einsum("ed,edf->ef", ns[src[eb]], a[eb])
        summed = np.zeros((len(host_nodes), D), np.float32)
        hn_index = {n: i for i, n in enumerate(host_nodes)}
        idx = np.fromiter((hn_index[n] for n in dst[eb]), np.int64, len(eb))
        np.add.at(summed, idx, msg)
        cnt = counts[big].astype(np.float32)[:, None]
        host_rows = np.maximum(summed / cnt + b[None, :], 0.0).astype(np.float32)

    meta = dict(
        n_nodes=n_nodes,
        TS=TS,
        nodes_k=nodes_k,
        flatslot=flatslot,
        host_nodes=host_nodes,
        host_rows=host_rows,
        bias=b,
    )
    return in_maps, meta


def _build(TS, enable_asserts=False, repeat=1):
    nc = bacc.Bacc(
        "TRN2",
        target_bir_lowering=False,
        debug=False,
        enable_asserts=enable_asserts,
        num_devices=NCORES,
    )
    axr_d = nc.dram_tensor("axr", [TS, 128, SB * TCOLS], BF16, kind="ExternalInput")
    meta_d = nc.dram_tensor("meta", [TS, 128, SB * MCOLS], F32, kind="ExternalInput")
    iota_d = nc.dram_tensor("iota", [128, NSLOT], F32, kind="ExternalInput")
    ob_d = nc.dram_tensor("onesbias", [1, 64], BF16, kind="ExternalInput")
    out_d = nc.dram_tensor("out", [TS, NSLOT, SB * D], F32, kind="ExternalOutput")

    with tile.TileContext(nc) as tc, ExitStack() as ctx:
        cpool = ctx.enter_context(tc.tile_pool(name="const", bufs=1))
        apool = ctx.enter_context(tc.tile_pool(name="apool", bufs=3))
        spool = ctx.enter_context(tc.tile_pool(name="spool", bufs=3))
        wpool = ctx.enter_context(tc.tile_pool(name="wpool", bufs=4))
        opool = ctx.enter_context(tc.tile_pool(name="opool", bufs=2))
        ps_m = ctx.enter_context(tc.tile_pool(name="ps_m", bufs=2, space="PSUM"))
        ps_s = ctx.enter_context(tc.tile_pool(name="ps_s", bufs=2, space="PSUM"))

        iota_t = cpool.tile([128, NSLOT], F32, tag="iota")
        nc.sync.dma_start(iota_t[:], iota_d[:])
        ob_t = cpool.tile([1, 64], BF16, tag="ob")
        nc.sync.dma_start(ob_t[:], ob_d[:])

        # Two persistent block-diagonal x operands (one per super parity);
        # off-diagonal cells zeroed once (spread copies only touch the
        # diagonal 32-row blocks, so reuse keeps them zero).
        xm = []
        for i in range(2):
            t_ = cpool.tile([128, SB, 4, GPT], BF16, tag=f"xmega{i}")
            nc.vector.memset(t_[:], 0.0)
            xm.append(t_)

        def dma_in(q):
            # split the big stream across two DGE queues (SP + ACT
            # sequencers) so more DMA engines run it in parallel
            t_ = apool.tile([128, SB * TCOLS], BF16, tag="axr")
            nc.sync.dma_start(t_[0:64], axr_d[q][0:64])
            nc.scalar.dma_start(t_[64:128], axr_d[q][64:128])
            m_ = spool.tile([128, SB * MCOLS], F32, tag="meta")
            nc.sync.dma_start(m_[:], meta_d[q])
            return t_, m_

        # repeat>1 unrolls the whole body again over the same inputs — a
        # timing-only variant so per-invocation device time can be read off
        # the slope of repeated-execute wall time (outputs are rewritten
        # with identical values each rep).
        qs = [q for _ in range(repeat) for q in range(TS)]
        at_buf = {0: dma_in(qs[0])}
        if len(qs) > 1:
            at_buf[1] = dma_in(qs[1])

        pend = None  # (s_t, msgTT, osup, s) of the previous tile

        def finish(pend_v):
            s_t, msgTT, osup_, s_ = pend_v
            sum_ps = ps_s.tile([NSLOT, D], F32, tag="sum")
            for cb in range(4):
                nc.tensor.matmul(
                    sum_ps[:],
                    s_t[:, cb, :],
                    msgTT[:, 32 * cb : 32 * cb + 32],
                    start=(cb == 0),
                    stop=False,
                )
            nc.tensor.matmul(
                sum_ps[:], ob_t[:, 0:32], ob_t[:, 32:64], start=False, stop=True
            )
            nc.scalar.activation(
                osup_[:, D * s_ : D * s_ + D],
                sum_ps[:],
                mybir.ActivationFunctionType.Relu,
            )

        osup = None
        for pos, q in enumerate(qs):
            at, mt = at_buf.pop(pos)
            if pos + 2 < len(qs):
                at_buf[pos + 2] = dma_in(qs[pos + 2])
            atv = at.rearrange("p (s c) -> p s c", s=SB)
            xv = xm[pos % 2]

            # Spread compact x columns into the block-diagonal operand
            # (column-only moves within each 32-partition slab; split
            # across DVE and ACT to balance engine load).
            for j in range(4):
                dst_ = xv[32 * j : 32 * j + 32, :, j, :]
                src_ = atv[32 * j : 32 * j + 32, :, ACOLS : ACOLS + XCOLS]
                if j < 2:
                    nc.vector.tensor_copy(dst_, src_)
                else:
                    nc.scalar.copy(dst_, src_)

            osup_prev = osup
            osup = opool.tile([NSLOT, SB * D], F32, tag="osup")

            for s in range(SB):
                # stage 1: 8 quad matmuls. lhsT = A4 [128,128] (4 groups'
                # A blocks; 128-col bf16 stationary -> compiler enables FWL,
                # 2x weight-load) x block-diag x [128,16] -> [128,16] whose
                # diagonal 32x4 blocks are the quad's msgT; garbage
                # elsewhere. The host-side rank permutation (row formula in
                # _prep) selects the useful rows after the block transpose.
                msgT_ps = ps_m.tile([128, 8, 16], F32, tag="msgT")
                for k in range(8):
                    nc.tensor.matmul(
                        msgT_ps[:, k, :],
                        atv[:, s, 128 * k : 128 * k + 128],
                        xv[:, s, :, 4 * k : 4 * k + 4],
                        start=True,
                        stop=True,
                    )
                # selector S'[r, cb, m] = (rank[r]==m) * recip[r]  (bf16);
                # split across DVE and GPSIMD to balance engine load
                s_t = spool.tile([128, 4, NSLOT], BF16, tag="S")
                for cb in range(4):
                    eng = nc.vector if cb < 2 else nc.gpsimd
                    eng.tensor_scalar(
                        s_t[:, cb, :],
                        iota_t[:],
                        mt[:, MCOLS * s + cb : MCOLS * s + cb + 1],
                        mt[:, MCOLS * s + 4 + cb : MCOLS * s + 5 + cb],
                        mybir.AluOpType.is_equal,
                        mybir.AluOpType.mult,
                    )
                # PSUM -> SBUF (bf16) then 32x32 block transpose on DVE
                msgT_sb = wpool.tile([128, EPT], BF16, tag="msgTsb")
                nc.scalar.copy(
                    msgT_sb[:], msgT_ps.rearrange("p a b -> p (a b)")
                )
                msgTT = wpool.tile([128, EPT], BF16, tag="msgTT")
                nc.vector.transpose(msgTT[:], msgT_sb[:])

                if pend is not None:
                    finish(pend)
                    if s == 0 and pos > 0:
                        nc.sync.dma_start(out_d[qs[pos - 1]], osup_prev[:])
                pend = (s_t, msgTT, osup, s)

        finish(pend)
        nc.sync.dma_start(out_d[qs[-1]], osup[:])

    nc.compile()
    return nc


_BUILD_CACHE = {}


def _built(TS):
    nc = _BUILD_CACHE.get(TS)
    if nc is None:
        nc = _build(TS)
        _BUILD_CACHE[TS] = nc
    return nc


def _finalize(results, meta):
    sup = np.concatenate([r["out"] for r in results], axis=0)  # [NC*TS,32,SB*D]
    ncts = sup.shape[0]
    rows = (
        sup.reshape(ncts, NSLOT, SB, D)
        .transpose(0, 2, 1, 3)                                 # [t', s, m, f]
        .reshape(-1, D)
    )
    b = meta["bias"]
    out = np.empty((meta["n_nodes"], D), np.float32)
    out[:] = np.maximum(b, 0.0)[None, :]
    out[meta["nodes_k"]] = rows[meta["flatslot"]]
    if meta["host_rows"] is not None:
        out[meta["host_nodes"]] = meta["host_rows"]
    return out


def kernel(node_states, edge_index, a_in, bias):
    in_maps, meta = _prep(node_states, edge_index, a_in, bias)
    nc = _built(meta["TS"])
    res = run_bass_kernel_spmd(nc, in_maps, list(range(NCORES)))
    return _finalize(res.results, meta)


if __name__ == "__main__":
    np.random.seed(0)
    n_nodes, n_edges = 700, 3000
    ns = np.random.randn(n_nodes, D).astype(np.float32)
    ei = np.random.randint(0, n_nodes, (n_edges, 2)).astype(np.int64)
    a = (np.random.randn(n_edges, D, D) / np.sqrt(D)).astype(np.float32)
    b = np.random.uniform(-0.2, 0.2, D).astype(np.float32)

    x_i = ns[ei[:, 0]]
    msg = np.einsum("ed,edf->ef", x_i, a)
    summed = np.zeros((n_nodes, D), np.float32)
    np.add.at(summed, ei[:, 1], msg)
    cnt = np.bincount(ei[:, 1], minlength=n_nodes).astype(np.float32)
    expected = np.maximum(summed / np.maximum(cnt, 1.0)[:, None] + b[None, :], 0.0)

    if os.environ.get("RUN_HW"):
        actual = kernel(ns, ei, a, b)
    else:
        from concourse.bass_interp import CoreSim

        in_maps, meta = _prep(ns, ei, a, b)
        nc = _build(meta["TS"], enable_asserts=True)
        outs = []
        for c in range(NCORES):
            sim = CoreSim(nc, trace=False)
            for k, v in in_maps[c].items():
                sim.tensor(k)[:] = v
            sim.simulate()
            outs.append({"out": np.array(sim.tensor("out"))})
        actual = _finalize(outs, meta)

    err = np.abs(actual - expected)
    denom = np.abs(expected).max()
    print("max abs err:", err.max(), "rel to scale:", err.max() / denom)
    rel = np.linalg.norm(actual - expected) / np.linalg.norm(expected)
    print("l2 rel:", rel)
    assert rel < 2e-2, "FAIL"
    print("PASS")
